# revision 7
# baseline (speedup 1.0000x reference)
"""Bahdanau additive attention on 8 Trainium2 NeuronCores.

Problem shapes (hardcoded): B=4, T=128, S=512, H=256, fp32.

Sharding: data-parallel over (batch, T-half): core c handles b = c//2,
t in [64*(c%2), 64*(c%2)+64).  Same SPMD program on every core; weights
replicated.  No collectives.

Algorithm: the additive-attention score
    e[t,s] = sum_h v[h] * tanh(pq[t,h] + pe[s,h])
is evaluated through a separable expansion instead of materializing the
(T,S,H) tensor.  With a = tanh(pq), w = tanh(pe):
    tanh(x+y) = (a+w)/(1+a*w)  ~=  tanh(x) + sum_{j=1..J} (c0_j a^{j-1}
                                   + c1_j a^{j+1}) w^j
(banded bivariate least-squares fit; the tanh(x) term is constant over s
and drops out under softmax shift-invariance).  Each term is a rank-1
update in (t,s) contracted over h, so e becomes J*HC=20 dense matmul
passes accumulated in one PSUM bank:
    e = sum_j M_j^T @ W_j,   M_j = v o a^{j-1} (c0_j + c1_j a^2),
                             W_j = w^j.
The w-power tiles come from a product DAG split across the Scalar
(Square), Vector, and GpSimd engines; the A-side chain tiles are small
(128x128).  Masking is one extra rank-1 pass adding -30 to masked s.
Softmax tail: exp on ACT with accum_out giving the row-sum Z for free,
PE transposes for alpha^T, one matmul for the context, fp16 throughout
with fp32 PSUM accumulation.
"""

import numpy as np

B, T, S, H = 4, 128, 512, 256
TLOC = 64
NCORES = 8
P = 128
HC = H // P        # 2 h-chunks
J = 9              # expansion order

# banded fit coefficients (see fit_final.py): relerr 6.1e-3 end-to-end
C0 = [1.003813, -0.94063, 0.484471, -2.849085, 7.735563,
      9.050273, -22.98368, -14.958482, 26.153909]
C1 = [-0.899529, 1.332728, -2.497427, -0.527733, 2.126473,
      -1.194412, 5.811916, 9.643296, -16.48554]

_CACHE = {}


def build_module():
    if "nc" in _CACHE:
        return _CACHE["nc"]

    try:
        import concourse.bass  # noqa: F401
    except ImportError:
        import sys
        sys.path.insert(0, "/opt/trn_rl_repo")

    import concourse.tile as tile
    from concourse import bacc, mybir

    f32 = mybir.dt.float32
    f16 = mybir.dt.float16
    AF = mybir.ActivationFunctionType
    ALU = mybir.AluOpType

    nc = bacc.Bacc(
        "TRN2",
        target_bir_lowering=False,
        debug=False,
        enable_asserts=False,
        num_devices=NCORES,
    )

    # packed fp16 inputs
    # pk_b: [qT (128) | wsT (512) | vbc (128) | ident (64)] (128 x 832)
    # pk_a: [encT (1024) | whT (512)]                       (128 x 1536)
    # pk_c: [ctx enc (1024) | woutT (1024) | mrhs (512)]    (128 x 2560)
    d_pa = nc.dram_tensor("pack_a", (P, 1024), f16, kind="ExternalInput").ap()
    d_pa2 = nc.dram_tensor("pack_a2", (P, 512), f16, kind="ExternalInput").ap()
    d_pb = nc.dram_tensor("pack_b", (P, 832), f16, kind="ExternalInput").ap()
    d_pc = nc.dram_tensor("pack_c", (P, 2560), f16, kind="ExternalInput").ap()
    d_out = nc.dram_tensor("out_l", (TLOC, H), f32, kind="ExternalOutput").ap()

    with tile.TileContext(nc) as tc:
        from contextlib import ExitStack

        with ExitStack() as ctx:
            consts = ctx.enter_context(tc.tile_pool(name="consts", bufs=1))
            bpow = ctx.enter_context(tc.tile_pool(name="bpow", bufs=1))
            asm = ctx.enter_context(tc.tile_pool(name="asm", bufs=1))
            tailp = ctx.enter_context(tc.tile_pool(name="tailp", bufs=1))
            psA = ctx.enter_context(tc.tile_pool(name="psA", bufs=1, space="PSUM"))
            psB = ctx.enter_context(tc.tile_pool(name="psB", bufs=1, space="PSUM"))
            psQ = ctx.enter_context(tc.tile_pool(name="psQ", bufs=1, space="PSUM"))
            psE = ctx.enter_context(tc.tile_pool(name="psE", bufs=1, space="PSUM"))
            psT = ctx.enter_context(tc.tile_pool(name="psT", bufs=3, space="PSUM"))

            pb = consts.tile([P, 832], f16)
            nc.sync.dma_start(pb[:], d_pb[:, :])
            pa = consts.tile([P, 1024], f16)
            nc.sync.dma_start(pa[:], d_pa[:, :])
            pa2 = consts.tile([P, 512], f16)
            nc.sync.dma_start(pa2[:], d_pa2[:, :])
            pc = consts.tile([P, 2560], f16)
            nc.sync.dma_start(pc[:], d_pc[:, :])

            encT = [pa[:, 0:512], pa2[:, 0:512]]            # (h-chunk, s)
            wh_sb = [pa[:, 512 + kc * H:512 + (kc + 1) * H] for kc in range(HC)]
            qT = pb[:, 0:128]                               # [hc0 t | hc1 t]
            ws_sb = [pb[:, 128 + kc * H:128 + (kc + 1) * H] for kc in range(HC)]
            vbc = pb[:, 640:768]
            ident = pb[:, 768:832]                          # rows 0:64 = I64
            ctxenc = pc[:, 0:1024]                          # 4 x (128 x 256)
            wout_sb = [pc[:, 1024 + fc * H:1024 + (fc + 1) * H] for fc in range(4)]
            mrhs = pc[:, 2048:2560]                         # (-30/128)*(1-mask)

            neg4 = consts.tile([TLOC, 1], f32)
            nc.vector.memset(neg4[:], -4.0)

            ones64 = consts.tile([P, TLOC], f16)
            nc.vector.memset(ones64[:], 1.0)

            # ---- projections (PE): pq first so alpha/a2/M-chain start early
            pq_ps = psQ.tile([P, 128], f32, name="pq_ps")
            for oc in range(HC):
                for kc in range(HC):
                    nc.tensor.matmul(
                        pq_ps[:, oc * TLOC:(oc + 1) * TLOC],
                        lhsT=ws_sb[kc][:, oc * P:(oc + 1) * P],
                        rhs=qT[:, kc * TLOC:(kc + 1) * TLOC],
                        start=(kc == 0), stop=(kc == HC - 1),
                    )
            pe_ps = [psA.tile([P, 512], f32, name="pe_ps0"),
                     psB.tile([P, 512], f32, name="pe_ps1")]
            for oc in range(HC):
                for kc in range(HC):
                    nc.tensor.matmul(
                        pe_ps[oc][:],
                        lhsT=wh_sb[kc][:, oc * P:(oc + 1) * P],
                        rhs=encT[kc][:],
                        start=(kc == 0), stop=(kc == HC - 1),
                    )

            # ---- base activations (ACT): alpha/a2 first, then w halves ----
            alpha = asm.tile([P, 128], f16, name="alpha")
            a2 = asm.tile([P, 128], f16, name="a2")
            with tc.high_priority():
                nc.scalar.activation(alpha[:], pq_ps[:], AF.Tanh)
                nc.scalar.activation(a2[:], alpha[:], AF.Square)
            w1 = bpow.tile([P, 1024], f16, name="w1")
            for oc in range(HC):
                nc.scalar.activation(w1[:, oc * 512:(oc + 1) * 512],
                                     pe_ps[oc][:], AF.Tanh)

            # ---- B-side power DAG ----
            # ACT: squares w2,w4,w8,w10; DVE: products w3,w5,w9,w6,w7 +
            # all G/At small tiles; GPS: the 10 small M multiplies only.
            Wt = {1: w1}
            for j in range(2, J + 1):
                Wt[j] = bpow.tile([P, 1024], f16, name=f"w{j}")
            At = {}
            At[0] = vbc
            for k in range(1, J):
                At[k] = asm.tile([P, 128], f16, name=f"At{k}")
            G = {}
            M = {}
            for j in range(1, J + 1):
                G[j] = asm.tile([P, 128], f16, name=f"G{j}")
                M[j] = asm.tile([P, 128], f16, name=f"M{j}")

            def mk_g(j, eng):
                eng.tensor_scalar(G[j][:], a2[:], float(C1[j - 1]),
                                  float(C0[j - 1]), ALU.mult, ALU.add)

            def mk_m(j, eng):
                eng.tensor_tensor(out=M[j][:], in0=At[j - 1][:], in1=G[j][:],
                                  op=ALU.mult)

            def mk_at(k, eng):
                src = At[k - 2] if k >= 2 else vbc
                other = a2 if k >= 2 else alpha
                eng.tensor_tensor(out=At[k][:], in0=src[:], in1=other[:],
                                  op=ALU.mult)

            def mk_w(j, a, b, eng):
                eng.tensor_tensor(out=Wt[j][:], in0=Wt[a][:], in1=Wt[b][:],
                                  op=ALU.mult)

            V = nc.vector
            GP = nc.gpsimd
            # DVE: first the G/At that gate early M's, then w products
            # interleaved with the rest; GPS does the M multiplies.
            mk_g(1, V); mk_at(1, V)
            mk_g(2, V); mk_at(2, V)
            mk_g(3, V); mk_at(3, V)
            mk_m(1, GP); mk_m(2, GP); mk_m(3, GP)
            nc.scalar.activation(Wt[2][:], w1[:], AF.Square)
            mk_w(3, 2, 1, V)
            mk_g(4, V); mk_at(4, V)
            mk_g(5, V); mk_at(5, V)
            mk_m(4, GP); mk_m(5, GP)
            nc.scalar.activation(Wt[4][:], Wt[2][:], AF.Square)
            mk_w(5, 4, 1, V)
            mk_g(6, V); mk_at(6, V)
            mk_g(7, V); mk_at(7, V)
            mk_m(6, GP); mk_m(7, GP)
            nc.scalar.activation(Wt[8][:], Wt[4][:], AF.Square)
            mk_w(9, 5, 4, V)
            mk_g(8, V); mk_at(8, V)
            mk_g(9, V)
            mk_m(8, GP); mk_m(9, GP)
            mk_w(6, 3, 3, V)
            mk_w(7, 4, 3, V)

            # ---- main accumulation: e = sum_j M_j^T W_j + mask ----
            e_ps = psE.tile([TLOC, 512], f32, name="e_ps")
            pass_order = [1, 2, 3, 4, 5, 8, 9, 6, 7]
            for n, j in enumerate(pass_order):
                for hc in range(HC):
                    nc.tensor.matmul(
                        e_ps[:],
                        lhsT=M[j][:, hc * TLOC:(hc + 1) * TLOC],
                        rhs=Wt[j][:, hc * 512:(hc + 1) * 512],
                        start=(n == 0 and hc == 0), stop=False,
                    )
            nc.tensor.matmul(e_ps[:], lhsT=ones64[:], rhs=mrhs[:],
                             start=False, stop=True)

            # ---- softmax tail ----
            pt = tailp.tile([TLOC, 512], f16, name="pt")
            zacc = tailp.tile([TLOC, 1], f32, name="zacc")
            nc.scalar.activation(pt[:], e_ps[:], AF.Exp,
                                 bias=neg4[:, 0:1], accum_out=zacc[:])
            r_sb = tailp.tile([TLOC, 1], f32, name="r_sb")
            nc.vector.reciprocal(r_sb[:], zacc[:])

            ptT_ps = psT.tile([P, 256], f16, tag="tail", name="ptT_ps")
            for sb in range(4):
                nc.tensor.transpose(
                    ptT_ps[:, sb * TLOC:(sb + 1) * TLOC],
                    pt[:, sb * P:(sb + 1) * P],
                    ident[0:TLOC, 0:TLOC],
                )
            ptT = tailp.tile([P, 256], f16, name="ptT")
            for sb in range(4):
                nc.vector.tensor_copy(ptT[:, sb * TLOC:(sb + 1) * TLOC],
                                      ptT_ps[:, sb * TLOC:(sb + 1) * TLOC])

            # q-side half of the output GEMM runs while exp/softmax happen
            attn_ps = psT.tile([TLOC, H], f32, tag="tail", name="attn_ps")
            nc.tensor.matmul(attn_ps[:], lhsT=qT[:, 0:TLOC],
                             rhs=wout_sb[0][:], start=True, stop=False)
            nc.tensor.matmul(attn_ps[:], lhsT=qT[:, TLOC:128],
                             rhs=wout_sb[1][:], start=False, stop=False)

            cun_ps = psT.tile([TLOC, H], f32, tag="tail", name="cun_ps")
            for sb in range(4):
                nc.tensor.matmul(
                    cun_ps[:],
                    lhsT=ptT[:, sb * TLOC:(sb + 1) * TLOC],
                    rhs=ctxenc[:, sb * H:(sb + 1) * H],
                    start=(sb == 0), stop=(sb == 3),
                )
            c_sb = tailp.tile([TLOC, H], f16, name="c_sb")
            ct_ps = psT.tile([P, 128], f16, tag="tail", name="ct_ps")
            ct_sb = tailp.tile([P, 128], f16, name="ct_sb")
            for i in range(HC):
                nc.vector.tensor_scalar_mul(c_sb[:, i * P:(i + 1) * P],
                                            cun_ps[:, i * P:(i + 1) * P],
                                            r_sb[:])
                nc.tensor.transpose(
                    ct_ps[:, i * TLOC:(i + 1) * TLOC],
                    c_sb[:, i * P:(i + 1) * P],
                    ident[0:TLOC, 0:TLOC],
                )
                nc.vector.tensor_copy(ct_sb[:, i * TLOC:(i + 1) * TLOC],
                                      ct_ps[:, i * TLOC:(i + 1) * TLOC])
                nc.tensor.matmul(attn_ps[:],
                                 lhsT=ct_sb[:, i * TLOC:(i + 1) * TLOC],
                                 rhs=wout_sb[2 + i][:],
                                 start=False, stop=(i == HC - 1))
            o_sb = tailp.tile([TLOC, H], f32, name="o_sb")
            nc.scalar.activation(o_sb[:], attn_ps[:], AF.Tanh)
            nc.sync.dma_start(d_out[:, :], o_sb[:])

    nc.compile()
    _CACHE["nc"] = nc
    return nc


def make_in_maps(query, encoder_outputs, src_lengths, Ws, Wh, v, Wout):
    h16 = np.float16
    wsT = np.asarray(Ws, h16).T
    whT = np.asarray(Wh, h16).T
    woutT = np.asarray(Wout, h16).T                  # (2H, H)
    sl = np.asarray(src_lengths)
    ident = np.eye(TLOC, dtype=h16)

    pack_a = np.zeros((NCORES, P, 1024), h16)
    pack_a2 = np.zeros((NCORES, P, 512), h16)
    pack_b = np.zeros((NCORES, P, 832), h16)
    pack_c = np.zeros((NCORES, P, 2560), h16)
    for c in range(NCORES):
        b, th = c // 2, c % 2
        t0 = th * TLOC
        encT = np.asarray(encoder_outputs[b], h16).T      # (H, S)
        enc = np.asarray(encoder_outputs[b], h16)         # (S, H)
        qTl = np.asarray(query[b, t0:t0 + TLOC, :], h16).T  # (H, TLOC)
        msk = (np.arange(S) < int(sl[b]))
        pack_a[c, :, 0:512] = encT[0:P]
        pack_a2[c, :, 0:512] = encT[P:2 * P]
        for kc in range(HC):
            pack_a[c, :, 512 + kc * H:512 + (kc + 1) * H] = \
                whT[kc * P:(kc + 1) * P]
            pack_b[c, :, kc * TLOC:(kc + 1) * TLOC] = qTl[kc * P:(kc + 1) * P]
            pack_b[c, :, 128 + kc * H:128 + (kc + 1) * H] = \
                wsT[kc * P:(kc + 1) * P]
            pack_b[c, :, 640 + kc * TLOC:640 + (kc + 1) * TLOC] = \
                np.asarray(v, np.float32)[kc * P:(kc + 1) * P, None].astype(h16)
        pack_b[c, 0:TLOC, 768:832] = ident
        pack_c[c, :, 2048:2560] = np.where(msk, 0.0, -30.0 / 128.0)[None, :]
        for sb in range(4):
            pack_c[c, :, sb * H:(sb + 1) * H] = enc[sb * P:(sb + 1) * P]
        for fc in range(4):
            pack_c[c, :, 1024 + fc * H:1024 + (fc + 1) * H] = \
                woutT[fc * P:(fc + 1) * P]
    return [{"pack_a": np.ascontiguousarray(pack_a[c]),
             "pack_a2": np.ascontiguousarray(pack_a2[c]),
             "pack_b": np.ascontiguousarray(pack_b[c]),
             "pack_c": np.ascontiguousarray(pack_c[c])}
            for c in range(NCORES)]


def kernel(query, encoder_outputs, src_lengths, Ws, Wh, v, Wout):
    from concourse.bass_utils import run_bass_kernel_spmd

    nc = build_module()
    in_maps = make_in_maps(query, encoder_outputs, src_lengths, Ws, Wh, v, Wout)
    res = run_bass_kernel_spmd(nc, in_maps, core_ids=list(range(NCORES))).results
    out = np.empty((B, T, H), np.float32)
    for c in range(NCORES):
        b, th = c // 2, c % 2
        t0 = th * TLOC
        out[b, t0:t0 + TLOC, :] = res[c]["out_l"]
    return out


# revision 8
# speedup vs baseline: 1.0923x; 1.0923x over previous
"""Bahdanau additive attention on 8 Trainium2 NeuronCores.

Problem shapes (hardcoded): B=4, T=128, S=512, H=256, fp32.

Sharding: data-parallel over (batch, T-half): core c handles b = c//2,
t in [64*(c%2), 64*(c%2)+64).  Same SPMD program on every core; weights
replicated.  No collectives.

Algorithm: the additive-attention score
    e[t,s] = sum_h v[h] * tanh(pq[t,h] + pe[s,h])
is evaluated through a separable expansion instead of materializing the
(T,S,H) tensor.  With a = tanh(pq), w = tanh(pe):
    tanh(x+y) = (a+w)/(1+a*w)  ~=  tanh(x) + sum_{j=1..J} (c0_j a^{j-1}
                                   + c1_j a^{j+1}) w^j
(banded bivariate least-squares fit; the tanh(x) term is constant over s
and drops out under softmax shift-invariance).  Each term is a rank-1
update in (t,s) contracted over h, so e becomes J*HC=20 dense matmul
passes accumulated in one PSUM bank:
    e = sum_j M_j^T @ W_j,   M_j = v o a^{j-1} (c0_j + c1_j a^2),
                             W_j = w^j.
The w-power tiles come from a product DAG split across the Scalar
(Square), Vector, and GpSimd engines; the A-side chain tiles are small
(128x128).  Masking is one extra rank-1 pass adding -30 to masked s.
Softmax tail: exp on ACT with accum_out giving the row-sum Z for free,
PE transposes for alpha^T, one matmul for the context, fp16 throughout
with fp32 PSUM accumulation.
"""

import numpy as np

B, T, S, H = 4, 128, 512, 256
TLOC = 64
NCORES = 8
P = 128
HC = H // P        # 2 h-chunks
J = 9              # expansion order

# banded fit coefficients (see fit_final.py): relerr 6.1e-3 end-to-end
C0 = [1.003813, -0.94063, 0.484471, -2.849085, 7.735563,
      9.050273, -22.98368, -14.958482, 26.153909]
C1 = [-0.899529, 1.332728, -2.497427, -0.527733, 2.126473,
      -1.194412, 5.811916, 9.643296, -16.48554]

_CACHE = {}


def build_module():
    if "nc" in _CACHE:
        return _CACHE["nc"]

    try:
        import concourse.bass  # noqa: F401
    except ImportError:
        import sys
        sys.path.insert(0, "/opt/trn_rl_repo")

    import concourse.tile as tile
    from concourse import bacc, mybir

    f32 = mybir.dt.float32
    f16 = mybir.dt.float16
    AF = mybir.ActivationFunctionType
    ALU = mybir.AluOpType

    nc = bacc.Bacc(
        "TRN2",
        target_bir_lowering=False,
        debug=False,
        enable_asserts=False,
        num_devices=NCORES,
    )

    # packed fp16 inputs
    # pk_b: [qT (128) | wsT (512) | vbc (128) | ident (64)] (128 x 832)
    # pk_a: [encT (1024) | whT (512)]                       (128 x 1536)
    # pk_c: [ctx enc (1024) | woutT (1024) | mrhs (512)]    (128 x 2560)
    d_pa = nc.dram_tensor("pack_a", (P, 1536), f16, kind="ExternalInput").ap()
    d_pb = nc.dram_tensor("pack_b", (P, 832), f16, kind="ExternalInput").ap()
    d_pc = nc.dram_tensor("pack_c", (P, 2560), f16, kind="ExternalInput").ap()
    d_out = nc.dram_tensor("out_l", (TLOC, H), f32, kind="ExternalOutput").ap()

    with tile.TileContext(nc) as tc:
        from contextlib import ExitStack

        with ExitStack() as ctx:
            consts = ctx.enter_context(tc.tile_pool(name="consts", bufs=1))
            bpow = ctx.enter_context(tc.tile_pool(name="bpow", bufs=1))
            asm = ctx.enter_context(tc.tile_pool(name="asm", bufs=1))
            tailp = ctx.enter_context(tc.tile_pool(name="tailp", bufs=1))
            psA = ctx.enter_context(tc.tile_pool(name="psA", bufs=1, space="PSUM"))
            psB = ctx.enter_context(tc.tile_pool(name="psB", bufs=1, space="PSUM"))
            psQ = ctx.enter_context(tc.tile_pool(name="psQ", bufs=1, space="PSUM"))
            psE = ctx.enter_context(tc.tile_pool(name="psE", bufs=1, space="PSUM"))
            psT = ctx.enter_context(tc.tile_pool(name="psT", bufs=3, space="PSUM"))

            pb = consts.tile([P, 832], f16)
            nc.sync.dma_start(pb[:], d_pb[:, :])
            pa = consts.tile([P, 1536], f16)
            nc.sync.dma_start(pa[:], d_pa[:, :])
            pc = consts.tile([P, 2560], f16)
            nc.sync.dma_start(pc[:], d_pc[:, :])

            encT = [pa[:, 0:512], pa[:, 512:1024]]          # (h-chunk, s)
            wh_sb = [pa[:, 1024 + kc * H:1024 + (kc + 1) * H] for kc in range(HC)]
            qT = pb[:, 0:128]                               # [hc0 t | hc1 t]
            ws_sb = [pb[:, 128 + kc * H:128 + (kc + 1) * H] for kc in range(HC)]
            vbc = pb[:, 640:768]
            ident = pb[:, 768:832]                          # rows 0:64 = I64
            ctxenc = pc[:, 0:1024]                          # 4 x (128 x 256)
            wout_sb = [pc[:, 1024 + fc * H:1024 + (fc + 1) * H] for fc in range(4)]
            mrhs = pc[:, 2048:2560]                         # (-30/128)*(1-mask)

            neg4 = consts.tile([TLOC, 1], f32)
            nc.vector.memset(neg4[:], -4.0)

            ones64 = consts.tile([P, TLOC], f16)
            nc.vector.memset(ones64[:], 1.0)

            # ---- projections (PE): pq first so alpha/a2/M-chain start early
            pq_ps = psQ.tile([P, 128], f32, name="pq_ps")
            for oc in range(HC):
                for kc in range(HC):
                    nc.tensor.matmul(
                        pq_ps[:, oc * TLOC:(oc + 1) * TLOC],
                        lhsT=ws_sb[kc][:, oc * P:(oc + 1) * P],
                        rhs=qT[:, kc * TLOC:(kc + 1) * TLOC],
                        start=(kc == 0), stop=(kc == HC - 1),
                    )
            pe_ps = [psA.tile([P, 512], f32, name="pe_ps0"),
                     psB.tile([P, 512], f32, name="pe_ps1")]
            for oc in range(HC):
                for kc in range(HC):
                    nc.tensor.matmul(
                        pe_ps[oc][:],
                        lhsT=wh_sb[kc][:, oc * P:(oc + 1) * P],
                        rhs=encT[kc][:],
                        start=(kc == 0), stop=(kc == HC - 1),
                    )

            # ---- base activations: alpha on ACT, a2 on DVE ----
            alpha = asm.tile([P, 128], f16, name="alpha")
            a2 = asm.tile([P, 128], f16, name="a2")
            with tc.high_priority():
                nc.scalar.activation(alpha[:], pq_ps[:], AF.Tanh)
                nc.vector.tensor_tensor(out=a2[:], in0=alpha[:], in1=alpha[:],
                                        op=mybir.AluOpType.mult)
            w1 = bpow.tile([P, 1024], f16, name="w1")
            for oc in range(HC):
                nc.scalar.activation(w1[:, oc * 512:(oc + 1) * 512],
                                     pe_ps[oc][:], AF.Tanh)

            # ---- B-side power DAG ----
            # ACT: squares w2,w4,w8,w10; DVE: products w3,w5,w9,w6,w7 +
            # all G/At small tiles; GPS: the 10 small M multiplies only.
            Wt = {1: w1}
            for j in range(2, J + 1):
                Wt[j] = bpow.tile([P, 1024], f16, name=f"w{j}")
            At = {}
            At[0] = vbc
            for k in range(1, J):
                At[k] = asm.tile([P, 128], f16, name=f"At{k}")
            G = {}
            M = {}
            for j in range(1, J + 1):
                G[j] = asm.tile([P, 128], f16, name=f"G{j}")
                M[j] = asm.tile([P, 128], f16, name=f"M{j}")

            def mk_g(j, eng):
                eng.tensor_scalar(G[j][:], a2[:], float(C1[j - 1]),
                                  float(C0[j - 1]), ALU.mult, ALU.add)

            def mk_m(j, eng):
                eng.tensor_tensor(out=M[j][:], in0=At[j - 1][:], in1=G[j][:],
                                  op=ALU.mult)

            def mk_at(k, eng):
                src = At[k - 2] if k >= 2 else vbc
                other = a2 if k >= 2 else alpha
                eng.tensor_tensor(out=At[k][:], in0=src[:], in1=other[:],
                                  op=ALU.mult)

            def mk_w(j, a, b, eng):
                eng.tensor_tensor(out=Wt[j][:], in0=Wt[a][:], in1=Wt[b][:],
                                  op=ALU.mult)

            V = nc.vector
            GP = nc.gpsimd

            def sqh(j, a, hc):
                nc.scalar.activation(Wt[j][:, hc * 512:(hc + 1) * 512],
                                     Wt[a][:, hc * 512:(hc + 1) * 512],
                                     AF.Square)

            def mulh(j, a, b, hc, eng):
                eng.tensor_tensor(out=Wt[j][:, hc * 512:(hc + 1) * 512],
                                  in0=Wt[a][:, hc * 512:(hc + 1) * 512],
                                  in1=Wt[b][:, hc * 512:(hc + 1) * 512],
                                  op=ALU.mult)

            # A-side smalls: all on DVE early (DVE is idle pre-ladder);
            # M multiplies on GPS.
            for j in range(1, J + 1):
                mk_g(j, V)
                if j < J:
                    mk_at(j, V)
                mk_m(j, GP)
            # per-hc square/product ladder: ACT does w2,w4,w8 halves,
            # DVE does w3,w5,w9,w6,w7 halves
            for hc in range(HC):
                sqh(2, 1, hc)
                mulh(3, 2, 1, hc, V)
                sqh(4, 2, hc)
                mulh(5, 4, 1, hc, V)
                sqh(8, 4, hc)
                mulh(9, 5, 4, hc, V)
                mulh(6, 3, 3, hc, V)
                mulh(7, 4, 3, hc, V)

            # ---- main accumulation: e = sum_j M_j^T W_j + mask ----
            e_ps = psE.tile([TLOC, 512], f32, name="e_ps")
            pass_order = [1, 2, 3, 4, 5, 8, 9, 6, 7]
            for n, j in enumerate(pass_order):
                for hc in range(HC):
                    nc.tensor.matmul(
                        e_ps[:],
                        lhsT=M[j][:, hc * TLOC:(hc + 1) * TLOC],
                        rhs=Wt[j][:, hc * 512:(hc + 1) * 512],
                        start=(n == 0 and hc == 0), stop=False,
                    )
            nc.tensor.matmul(e_ps[:], lhsT=ones64[:], rhs=mrhs[:],
                             start=False, stop=True)

            # ---- softmax tail ----
            pt = tailp.tile([TLOC, 512], f16, name="pt")
            zacc = tailp.tile([TLOC, 1], f32, name="zacc")
            nc.scalar.activation(pt[:], e_ps[:], AF.Exp,
                                 bias=neg4[:, 0:1], accum_out=zacc[:])
            r_sb = tailp.tile([TLOC, 1], f32, name="r_sb")
            nc.vector.reciprocal(r_sb[:], zacc[:])

            ptT_ps = psT.tile([P, 256], f16, tag="tail", name="ptT_ps")
            for sb in range(4):
                nc.tensor.transpose(
                    ptT_ps[:, sb * TLOC:(sb + 1) * TLOC],
                    pt[:, sb * P:(sb + 1) * P],
                    ident[0:TLOC, 0:TLOC],
                )
            ptT = tailp.tile([P, 256], f16, name="ptT")
            for sb in range(4):
                nc.vector.tensor_copy(ptT[:, sb * TLOC:(sb + 1) * TLOC],
                                      ptT_ps[:, sb * TLOC:(sb + 1) * TLOC])

            # q-side half of the output GEMM runs while exp/softmax happen
            attn_ps = psT.tile([TLOC, H], f32, tag="tail", name="attn_ps")
            nc.tensor.matmul(attn_ps[:], lhsT=qT[:, 0:TLOC],
                             rhs=wout_sb[0][:], start=True, stop=False)
            nc.tensor.matmul(attn_ps[:], lhsT=qT[:, TLOC:128],
                             rhs=wout_sb[1][:], start=False, stop=False)

            cun_ps = psT.tile([TLOC, H], f32, tag="tail", name="cun_ps")
            for sb in range(4):
                nc.tensor.matmul(
                    cun_ps[:],
                    lhsT=ptT[:, sb * TLOC:(sb + 1) * TLOC],
                    rhs=ctxenc[:, sb * H:(sb + 1) * H],
                    start=(sb == 0), stop=(sb == 3),
                )
            c_sb = tailp.tile([TLOC, H], f16, name="c_sb")
            ct_ps = psT.tile([P, 128], f16, tag="tail", name="ct_ps")
            ct_sb = tailp.tile([P, 128], f16, name="ct_sb")
            for i in range(HC):
                nc.vector.tensor_scalar_mul(c_sb[:, i * P:(i + 1) * P],
                                            cun_ps[:, i * P:(i + 1) * P],
                                            r_sb[:])
                nc.tensor.transpose(
                    ct_ps[:, i * TLOC:(i + 1) * TLOC],
                    c_sb[:, i * P:(i + 1) * P],
                    ident[0:TLOC, 0:TLOC],
                )
                nc.vector.tensor_copy(ct_sb[:, i * TLOC:(i + 1) * TLOC],
                                      ct_ps[:, i * TLOC:(i + 1) * TLOC])
                nc.tensor.matmul(attn_ps[:],
                                 lhsT=ct_sb[:, i * TLOC:(i + 1) * TLOC],
                                 rhs=wout_sb[2 + i][:],
                                 start=False, stop=(i == HC - 1))
            o_sb = tailp.tile([TLOC, H], f32, name="o_sb")
            nc.scalar.activation(o_sb[:], attn_ps[:], AF.Tanh)
            nc.sync.dma_start(d_out[:, :], o_sb[:])

    nc.compile()
    _CACHE["nc"] = nc
    return nc


def make_in_maps(query, encoder_outputs, src_lengths, Ws, Wh, v, Wout):
    h16 = np.float16
    wsT = np.asarray(Ws, h16).T
    whT = np.asarray(Wh, h16).T
    woutT = np.asarray(Wout, h16).T                  # (2H, H)
    sl = np.asarray(src_lengths)
    ident = np.eye(TLOC, dtype=h16)

    pack_a = np.zeros((NCORES, P, 1536), h16)
    pack_b = np.zeros((NCORES, P, 832), h16)
    pack_c = np.zeros((NCORES, P, 2560), h16)
    for c in range(NCORES):
        b, th = c // 2, c % 2
        t0 = th * TLOC
        encT = np.asarray(encoder_outputs[b], h16).T      # (H, S)
        enc = np.asarray(encoder_outputs[b], h16)         # (S, H)
        qTl = np.asarray(query[b, t0:t0 + TLOC, :], h16).T  # (H, TLOC)
        msk = (np.arange(S) < int(sl[b]))
        for kc in range(HC):
            pack_a[c, :, kc * 512:(kc + 1) * 512] = encT[kc * P:(kc + 1) * P]
            pack_a[c, :, 1024 + kc * H:1024 + (kc + 1) * H] = \
                whT[kc * P:(kc + 1) * P]
            pack_b[c, :, kc * TLOC:(kc + 1) * TLOC] = qTl[kc * P:(kc + 1) * P]
            pack_b[c, :, 128 + kc * H:128 + (kc + 1) * H] = \
                wsT[kc * P:(kc + 1) * P]
            pack_b[c, :, 640 + kc * TLOC:640 + (kc + 1) * TLOC] = \
                np.asarray(v, np.float32)[kc * P:(kc + 1) * P, None].astype(h16)
        pack_b[c, 0:TLOC, 768:832] = ident
        pack_c[c, :, 2048:2560] = np.where(msk, 0.0, -30.0 / 128.0)[None, :]
        for sb in range(4):
            pack_c[c, :, sb * H:(sb + 1) * H] = enc[sb * P:(sb + 1) * P]
        for fc in range(4):
            pack_c[c, :, 1024 + fc * H:1024 + (fc + 1) * H] = \
                woutT[fc * P:(fc + 1) * P]
    return [{"pack_a": np.ascontiguousarray(pack_a[c]),
             "pack_b": np.ascontiguousarray(pack_b[c]),
             "pack_c": np.ascontiguousarray(pack_c[c])}
            for c in range(NCORES)]


def kernel(query, encoder_outputs, src_lengths, Ws, Wh, v, Wout):
    from concourse.bass_utils import run_bass_kernel_spmd

    nc = build_module()
    in_maps = make_in_maps(query, encoder_outputs, src_lengths, Ws, Wh, v, Wout)
    res = run_bass_kernel_spmd(nc, in_maps, core_ids=list(range(NCORES))).results
    out = np.empty((B, T, H), np.float32)
    for c in range(NCORES):
        b, th = c // 2, c % 2
        t0 = th * TLOC
        out[b, t0:t0 + TLOC, :] = res[c]["out_l"]
    return out
